# revision 3
# baseline (speedup 1.0000x reference)
"""Mixture-of-Softmax loss kernel for 8 Trainium2 NeuronCores.

out[s,v] = logsumexp_k( log_softmax_v(logits[s,k,v]) + log pi[s,k] )
         = log( sum_k pi[s,k] * exp(logits[s,k,v]) / Z[s,k] )

Sharding: vocab dimension of weight_matrix split across 8 cores. Per-core
logical shard width VS=6283 (V=50257 -> 8*6283=50264), padded on-chip to
VSP=6288 (= 12*512 + 144, multiple of 16 for fp8 DoubleRow APs) with zero
weight columns. Pad columns contribute exp(0)=1 to the local sum-of-exp and
are subtracted via the per-core `corr` input, then dropped on gather.

The big [S,K,V] logits matmul runs in fp8-e4m3 with perf_mode=DoubleRow
(2 contraction rows per PE pass). weight_matrix is scaled by 256 on the host
before the fp8 cast (its std is 0.02, below e4m3's min normal) and the Exp
activation un-scales with its free affine (scale=1/256). projT is cast to
fp8 on device after phase 0. Logits are small (|l| < ~4) so no max
subtraction is needed for a stable sum-of-exp in fp32.

Per core, per 128-token s-tile:
  PE   : logits[k] = projT[k]^T @ WT    (fp8 DoubleRow, fp32 PSUM)
  ACT  : E = exp(logits/256) (fp16) in 2048-wide reads across 4 PSUM banks,
         accum_out = per-group sums
  CC   : AllReduce(add) of local [128,2] sum-of-exp -> global Z
  DVE  : w_k = pi_k / Z_k ;  t = E0*(w0/w1) + E1   (one fused fp16 pass)
  ACT  : out = Ln(t * w1)  (one 6288-wide fp16 pass)
"""

import math
import os
import sys

import numpy as np

for _p in ("/opt/trn_rl_repo", "/opt/trn_rl_repo/concourse"):
    if os.path.isdir(_p) and _p not in sys.path:
        sys.path.insert(0, _p)

import ml_dtypes

import concourse.bacc as bacc
import concourse.hw_specs as hw_specs
import concourse.tile as tile
from concourse import mybir
from concourse.bass_utils import run_bass_kernel_spmd

# --- Activation-table patch -------------------------------------------------
# This kernel interleaves Exp (sum-of-exp pass) and Ln (output pass) on the
# scalar engine. The default table chooser assigns Exp -> "exp_and_others"
# and Ln -> "natural_log", causing a ~2.7us ACT_TABLE_LOAD on every switch.
# The "natural_log_exp_and_others" set contains BOTH functions; hide Exp/Ln
# from every other set so the chooser must use the combined set, making the
# table resident for the whole kernel.
_orig_get_activation_tables = hw_specs.get_activation_tables


def _patched_get_activation_tables(module_arch):
    tabs = _orig_get_activation_tables(module_arch)
    E = mybir.ActivationFunctionType.Exp
    L = mybir.ActivationFunctionType.Ln
    out = {}
    for name, funcs in tabs.items():
        if name != "natural_log_exp_and_others" and (E in funcs or L in funcs):
            funcs = funcs - {E, L}
        out[name] = funcs
    return out


bacc.get_activation_tables = _patched_get_activation_tables
# ---------------------------------------------------------------------------

BF16 = mybir.dt.bfloat16
FP16 = mybir.dt.float16
FP32 = mybir.dt.float32
FP8 = mybir.dt.float8e4
P = 128  # partitions
W_SCALE = 256.0  # host-side weight_matrix scale before fp8 cast


def _ceil_div(a, b):
    return (a + b - 1) // b


def build_program(n_cores=8, S=2048, D=1024, VSP=6288, KM=2, e_dtype=FP16,
                  use_collectives=True, reps=1, ln_func=None):
    """Build the SPMD Bass program (same program on all cores).

    Inputs (per core):
      hiddenT  [D, S]   bf16   (same on all cores)
      hiddenTs [D, S/n] bf16   (this core's token slice)
      w_projT  [D, KM*D] bf16  (same on all cores)
      w_gateT  [D, KM]  bf16   (same on all cores)
      wt       [D, VSP] fp8e4  (core's vocab shard of weight_matrix^T * 256)
      corr     [P, 1]   f32    (number of zero-pad columns in this shard)
    Output (per core):
      out      [S, VSP] fp16
    """
    DC = D // P           # contraction chunks (128 rows each)
    NDP = DC // 2         # DoubleRow pairs (256 rows each)
    ST = S // P           # token tiles
    J = KM * D
    JT = J // P           # projT row tiles
    DR = mybir.MatmulPerfMode.DoubleRow
    # vocab groups, one 4-bank PSUM tile each. Near-even sizes so the short
    # last chunk's matmuls still cover the next DoubleRow LDWEIGHTS (~366ns):
    # a 144-wide-only group would stall PE ~300ns per dpair.
    base = (VSP // 4) // 512 * 512
    groups = []
    v0 = 0
    for g in range(4):
        gw = base if g < 3 else VSP - 3 * base
        groups.append((v0, gw))
        v0 += gw
    assert v0 == VSP and groups[-1][1] <= 2048
    NG = len(groups)
    RG = [list(range(n_cores))]
    if ln_func is None:
        ln_func = mybir.ActivationFunctionType.Ln

    nc = bacc.Bacc(
        "TRN2",
        target_bir_lowering=False,
        debug=False,
        num_devices=n_cores,
    )

    hiddenT = nc.dram_tensor("hiddenT", [D, S], BF16, kind="ExternalInput").ap()
    hiddenTs = nc.dram_tensor(
        "hiddenTs", [D, S // n_cores], BF16, kind="ExternalInput"
    ).ap()
    w_projT = nc.dram_tensor("w_projT", [D, J], BF16, kind="ExternalInput").ap()
    w_gateT = nc.dram_tensor("w_gateT", [D, KM], BF16, kind="ExternalInput").ap()
    wt = nc.dram_tensor("wt", [D, VSP], FP8, kind="ExternalInput").ap()
    corr = nc.dram_tensor("corr", [P, 1], FP32, kind="ExternalInput").ap()
    out = nc.dram_tensor("out", [S, VSP], FP16, kind="ExternalOutput").ap()

    ht_r = hiddenT.rearrange("(c p) s -> c p s", p=P)
    hts_r = hiddenTs.rearrange("(c p) s -> c p s", p=P)
    wp_r = w_projT.rearrange("(c p) j -> c p j", p=P)
    wg_r = w_gateT.rearrange("(c p) k -> c p k", p=P)
    wt_r = wt.rearrange("(c p) v -> c p v", p=P)

    def emit_once(tc):
        with (
            tc.tile_pool(name="singles", bufs=1) as singles,
            tc.tile_pool(name="gates", bufs=ST) as gates,
            tc.tile_pool(name="dram", bufs=1, space="DRAM") as dpool,
            tc.tile_pool(name="pj", bufs=3) as pjp,
        ):
            PJ_PRELOAD = 3

            def load_pj(i):
                srow = i * P
                ci = srow // SSH
                soff = srow - ci * SSH
                PJ = pjp.tile([P, JT, P], FP8, tag="PJ", name=f"PJ_{i}")
                nc.sync.dma_start(
                    out=PJ,
                    in_=proj_ag[ci][:, :, soff:soff + P].rearrange(
                        "t p s -> p t s"
                    ),
                )
                return PJ

            # Resident fp8 vocab-shard weights, one tile per 2048-wide group
            # so the first matmuls only wait on their own slice of the load.
            WTs = []
            for gi, (v0, gw) in enumerate(groups):
                wt_tile = singles.tile([P, DC, gw], FP8, tag=f"wt{gi}",
                                       name=f"WT_{gi}")
                for c in range(DC):
                    nc.sync.dma_start(out=wt_tile[:, c, :],
                                      in_=wt_r[c][:, v0:v0 + gw])
                WTs.append(wt_tile)
            corr_sb = singles.tile([P, 1], FP32)
            nc.sync.dma_start(out=corr_sb, in_=corr)

            # Phase 0 is sharded over cores: each core computes projT for
            # S/n_cores tokens, then an AllGather replicates the full projT
            # (in fp8 -- the main loop consumes fp8 anyway).
            SSH = S // n_cores  # tokens per core in phase 0
            assert SSH % P == 0 or n_cores == 1
            proj_in = dpool.tile([JT, P, SSH], FP8, name="proj_in")
            cc_addr = "Shared" if n_cores > 4 else "Local"
            proj_ag = dpool.tile([n_cores, JT, P, SSH], FP8, name="proj_ag",
                                 addr_space=cc_addr)
            ge_tiles = []
            rse_tiles = []

            # ACT-order chain: order-only edges keep the scalar engine's
            # instruction stream in emission order so Exp/Ln stay batched.
            last_act = [None]

            def act_chain(inst):
                if last_act[0] is not None:
                    tile.add_dep_helper(inst.ins, last_act[0].ins, sync=False,
                                        reason="act table batching")
                last_act[0] = inst
                return inst

            # ---------------- Phase 0: projT = (hidden @ w_proj^T)^T, gate ----
            with (
                tc.tile_pool(name="ph0", bufs=1) as ph0,
                tc.tile_pool(name="ph0ps", bufs=4, space="PSUM") as ps0,
                tc.tile_pool(name="ph0gps", bufs=2, space="PSUM") as gps0,
                tc.tile_pool(name="ph0st", bufs=4) as stg,
            ):
                HT = ph0.tile([P, DC, S], BF16)
                HTS = ph0.tile([P, DC, SSH], BF16)
                WP = ph0.tile([P, DC, J], BF16)
                WG = ph0.tile([P, DC, KM], BF16)
                for c in range(DC):
                    nc.sync.dma_start(out=HTS[:, c, :], in_=hts_r[c])
                    nc.sync.dma_start(out=WP[:, c, :], in_=wp_r[c])
                    nc.sync.dma_start(out=WG[:, c, :], in_=wg_r[c])
                    nc.sync.dma_start(out=HT[:, c, :], in_=ht_r[c])

                # projT[j, s] = sum_d w_projT[d, j] * hiddenT[d, s], for
                # this core's S/n_cores token slice; AllGather replicates.
                pj_tiles = {}
                PSC = min(512, SSH)
                for t in range(JT):
                    for s0 in range(0, SSH, PSC):
                        sw = min(PSC, SSH - s0)
                        psum = ps0.tile([P, PSC], FP32, tag="mm")
                        for d in range(DC):
                            nc.tensor.matmul(
                                psum[:, :sw],
                                lhsT=WP[:, d, t * P:(t + 1) * P],
                                rhs=HTS[:, d, s0:s0 + sw],
                                start=(d == 0),
                                stop=(d == DC - 1),
                            )
                        st = stg.tile([P, PSC], FP8, tag="st")
                        nc.vector.tensor_copy(st[:, :sw], psum[:, :sw])
                        nc.sync.dma_start(out=proj_in[t, :, s0:s0 + sw],
                                          in_=st[:, :sw])
                if use_collectives:
                    nc.gpsimd.collective_compute(
                        "AllGather",
                        mybir.AluOpType.bypass,
                        replica_groups=RG,
                        ins=[proj_in.opt()],
                        outs=[proj_ag.opt()],
                    )
                else:
                    nc.sync.dma_start(out=proj_ag[0], in_=proj_in[:])
                # Prefetch the first main-loop lhsT slices now so their
                # DMAs aren't queued behind the rest of phase 0.
                for i in range(min(PJ_PRELOAD, ST)):
                    pj_tiles[i] = load_pj(i)

                # gate logits -> pi (unnormalized e, and 1/sum_e)
                for i in range(ST):
                    gp = gps0.tile([P, KM], FP32, tag="g")
                    for d in range(DC):
                        nc.tensor.matmul(
                            gp,
                            lhsT=HT[:, d, i * P:(i + 1) * P],
                            rhs=WG[:, d, :],
                            start=(d == 0),
                            stop=(d == DC - 1),
                        )
                    negm = gates.tile([P, 1], FP32, tag="negm")
                    nc.vector.reduce_max(
                        out=negm, in_=gp, axis=mybir.AxisListType.X, negate=True
                    )
                    ge = gates.tile([P, KM], FP32, tag="ge")
                    se = gates.tile([P, 1], FP32, tag="se")
                    act_chain(nc.scalar.activation(
                        out=ge, in_=gp, func=mybir.ActivationFunctionType.Exp,
                        bias=negm, accum_out=se,
                    ))
                    rse = gates.tile([P, 1], FP32, tag="rse")
                    nc.vector.reciprocal(rse, se)
                    ge_tiles.append(ge)
                    rse_tiles.append(rse)

            # ---------------- Main loop over token tiles ----------------------
            with (
                tc.tile_pool(name="ebuf", bufs=3) as ep,
                tc.tile_pool(name="zp", bufs=3) as zpp,
                tc.tile_pool(name="mmps", bufs=2, space="PSUM") as psm,
                tc.tile_pool(name="ocp", bufs=2) as ocp,
                tc.tile_pool(name="ttp", bufs=2) as ttp,
                tc.tile_pool(name="s2", bufs=3) as s2p,
                tc.tile_pool(name="cc", bufs=2 * ST, space="DRAM") as ccp,
            ):
                def emit_exps(i, k, E, zpart, PJ):
                    for g, (v0, gw) in enumerate(groups):
                        ps = psm.tile([P, 2048], FP32, tag="mm")
                        nchunks = _ceil_div(gw, 512)
                        for j in range(NDP):
                            lhsT = PJ[:, k * DC + 2 * j:k * DC + 2 * j + 2, :]
                            for c in range(nchunks):
                                cw = min(512, gw - c * 512)
                                nc.tensor.matmul(
                                    ps[:, c * 512:c * 512 + cw],
                                    lhsT=lhsT,
                                    rhs=WTs[g][:, 2 * j:2 * j + 2,
                                               c * 512:c * 512 + cw],
                                    start=(j == 0),
                                    stop=(j == NDP - 1),
                                    perf_mode=DR,
                                )
                        act_chain(nc.scalar.activation(
                            out=E[:, k, v0:v0 + gw],
                            in_=ps[:, :gw],
                            func=mybir.ActivationFunctionType.Exp,
                            scale=1.0 / W_SCALE,
                            accum_out=zpart[:, k, g:g + 1],
                        ))

                def emit_stage2(i, E, Zg):
                    srow = i * P
                    # w_k = pi_k / Z_k = ge_k * rse / Z_k
                    rz = s2p.tile([P, KM], FP32, tag="rz")
                    nc.vector.reciprocal(rz, Zg)
                    rzs = s2p.tile([P, KM], FP32, tag="rzs")
                    nc.vector.tensor_scalar_mul(rzs, rz, rse_tiles[i])
                    wk = s2p.tile([P, KM], FP32, tag="wk")
                    nc.vector.tensor_mul(wk, ge_tiles[i], rzs)
                    rw1 = s2p.tile([P, 1], FP32, tag="rw1")
                    nc.vector.reciprocal(rw1, wk[:, 1:2])
                    r01 = s2p.tile([P, 1], FP32, tag="r01")
                    nc.vector.tensor_mul(r01, wk[:, 0:1], rw1)
                    # t = E0 * (w0/w1) + E1, one fused DVE pass in fp16
                    t = ttp.tile([P, VSP], FP16, tag="t")
                    nc.vector.scalar_tensor_tensor(
                        out=t,
                        in0=E[:, 0, :],
                        scalar=r01,
                        in1=E[:, 1, :],
                        op0=mybir.AluOpType.mult,
                        op1=mybir.AluOpType.add,
                    )
                    oc = ocp.tile([P, VSP], FP16, tag="oc")
                    act_chain(nc.scalar.activation(
                        out=oc,
                        in_=t,
                        func=ln_func,
                        scale=wk[:, 1:2],
                    ))
                    nc.sync.dma_start(out=out[srow:srow + P, :], in_=oc)

                pending = []  # [(i, E, Zg)] awaiting stage 2 (depth 2)
                for i in range(ST):
                    if i not in pj_tiles:
                        pj_tiles[i] = load_pj(i)
                    nxt = i + PJ_PRELOAD
                    if nxt < ST and nxt not in pj_tiles:
                        pj_tiles[nxt] = load_pj(nxt)
                    PJ = pj_tiles.pop(i)
                    E = ep.tile([P, KM, VSP], e_dtype)
                    zpart = zpp.tile([P, KM, NG], FP32)
                    emit_exps(i, 0, E, zpart, PJ)
                    if len(pending) >= 2:
                        emit_stage2(*pending.pop(0))
                    for k in range(1, KM):
                        emit_exps(i, k, E, zpart, PJ)
                    zloc = s2p.tile([P, KM], FP32, tag="zloc")
                    for k in range(KM):
                        nc.vector.reduce_sum(
                            out=zloc[:, k:k + 1],
                            in_=zpart[:, k, :],
                            axis=mybir.AxisListType.X,
                        )
                    # remove pad-column contribution (exp(0)=1 per pad col)
                    nc.vector.tensor_scalar_sub(zloc, zloc, corr_sb)

                    cin = ccp.tile([P, KM], FP32, tag="cin")
                    cout = ccp.tile([P, KM], FP32, tag="cout",
                                    addr_space=cc_addr)
                    nc.sync.dma_start(out=cin, in_=zloc)
                    if use_collectives:
                        nc.gpsimd.collective_compute(
                            "AllReduce",
                            mybir.AluOpType.add,
                            replica_groups=RG,
                            ins=[cin.opt()],
                            outs=[cout.opt()],
                        )
                    else:
                        nc.sync.dma_start(out=cout, in_=cin)
                    Zg = s2p.tile([P, KM], FP32, tag="zg")
                    nc.sync.dma_start(out=Zg, in_=cout)
                    pending.append((i, E, Zg))
                while pending:
                    emit_stage2(*pending.pop(0))

    with tile.TileContext(nc) as tc:
        for _ in range(reps):
            emit_once(tc)

    nc.compile()
    return nc


def prep_inputs(hidden, weight_matrix, w_proj, w_gate, n_cores=8):
    """Host-side shard/transpose/cast. Returns (in_maps, VS, VSP)."""
    bf16 = ml_dtypes.bfloat16
    fp8 = ml_dtypes.float8_e4m3
    B, S, D = hidden.shape
    V = weight_matrix.shape[0]
    VS = _ceil_div(V, n_cores)       # logical shard width (6283)
    VSP = _ceil_div(VS, 16) * 16     # on-chip width, multiple of 16 (6288)

    hiddenT = np.ascontiguousarray(
        np.asarray(hidden, dtype=np.float32).reshape(S, D).T
    ).astype(bf16)
    w_projT = np.ascontiguousarray(
        np.asarray(w_proj, dtype=np.float32).T
    ).astype(bf16)
    w_gateT = np.ascontiguousarray(
        np.asarray(w_gate, dtype=np.float32).T
    ).astype(bf16)

    wmat = np.asarray(weight_matrix, dtype=np.float32)
    SSH = S // n_cores
    in_maps = []
    for c in range(n_cores):
        lo = c * VS
        hi = min(lo + VS, V)
        shard = np.zeros((VSP, D), dtype=np.float32)
        shard[: hi - lo] = wmat[lo:hi]
        wt_c = np.clip(
            np.ascontiguousarray(shard.T) * W_SCALE, -240.0, 240.0
        ).astype(fp8)
        npad = VSP - (hi - lo)
        corr_c = np.full((P, 1), float(npad), dtype=np.float32)
        in_maps.append(
            {
                "hiddenT": hiddenT,
                "hiddenTs": np.ascontiguousarray(
                    hiddenT[:, c * SSH:(c + 1) * SSH]
                ),
                "w_projT": w_projT,
                "w_gateT": w_gateT,
                "wt": wt_c,
                "corr": corr_c,
            }
        )
    return in_maps, VS, VSP


_PROGRAM_CACHE = {}


def kernel(hidden, weight_matrix, w_proj, w_gate):
    import time

    n_cores = 8
    B, S, D = hidden.shape
    V = weight_matrix.shape[0]
    KM = w_gate.shape[0]
    in_maps, VS, VSP = prep_inputs(hidden, weight_matrix, w_proj, w_gate,
                                   n_cores)

    key = (n_cores, S, D, VSP, KM)
    if key not in _PROGRAM_CACHE:
        _PROGRAM_CACHE[key] = build_program(n_cores, S, D, VSP, KM)
    nc = _PROGRAM_CACHE[key]

    # The axon terminal occasionally reports a transient
    # NRT_EXEC_UNIT_UNRECOVERABLE right after another process released the
    # devices; one retry after a pause usually succeeds.
    last_err = None
    for attempt in range(2):
        try:
            res = run_bass_kernel_spmd(nc, in_maps, core_ids=list(range(n_cores)))
            break
        except Exception as e:  # noqa: BLE001
            last_err = e
            time.sleep(15)
    else:
        raise last_err

    full = np.empty((S, VS * n_cores), dtype=np.float32)
    for c in range(n_cores):
        full[:, c * VS:(c + 1) * VS] = res.results[c]["out"][:, :VS]
    return full[:, :V].reshape(B, S, V)


# revision 16
# speedup vs baseline: 1.1968x; 1.1968x over previous
"""Mixture-of-Softmax loss kernel for 8 Trainium2 NeuronCores.

out[s,v] = logsumexp_k( log_softmax_v(logits[s,k,v]) + log pi[s,k] )
         = log( sum_k pi[s,k] * exp(logits[s,k,v]) / Z[s,k] )

Sharding: vocab dimension of weight_matrix split across 8 cores. Per-core
logical shard width VS=6283 (V=50257 -> 8*6283=50264), padded on-chip to
VSP=6288 (= 12*512 + 144, multiple of 16 for fp8 DoubleRow APs) with zero
weight columns. Pad columns contribute exp(0)=1 to the local sum-of-exp and
are subtracted via the per-core `corr` input, then dropped on gather.

The big [S,K,V] logits matmul runs in fp8-e4m3 with perf_mode=DoubleRow
(2 contraction rows per PE pass). weight_matrix is scaled by 256 on the host
before the fp8 cast (its std is 0.02, below e4m3's min normal) and the Exp
activation un-scales with its free affine (scale=1/256). projT is cast to
fp8 on device after phase 0. Logits are small (|l| < ~4) so no max
subtraction is needed for a stable sum-of-exp in fp32.

Per core, per 128-token s-tile:
  PE   : logits[k] = projT[k]^T @ WT    (fp8 DoubleRow, fp32 PSUM)
  ACT  : E = exp(logits/256) (fp16) in 2048-wide reads across 4 PSUM banks,
         accum_out = per-group sums
  CC   : AllReduce(add) of local [128,2] sum-of-exp -> global Z
  DVE  : w_k = pi_k / Z_k ;  t = E0*(w0/w1) + E1   (one fused fp16 pass)
  ACT  : out = Ln(t * w1)  (one 6288-wide fp16 pass)
"""

import math
import os
import sys

import numpy as np

for _p in ("/opt/trn_rl_repo", "/opt/trn_rl_repo/concourse"):
    if os.path.isdir(_p) and _p not in sys.path:
        sys.path.insert(0, _p)

import ml_dtypes

import concourse.bacc as bacc
import concourse.hw_specs as hw_specs
import concourse.tile as tile
from concourse import mybir
from concourse.bass_utils import run_bass_kernel_spmd

# --- Activation-table patch -------------------------------------------------
# This kernel interleaves Exp (sum-of-exp pass) and Ln (output pass) on the
# scalar engine. The default table chooser assigns Exp -> "exp_and_others"
# and Ln -> "natural_log", causing a ~2.7us ACT_TABLE_LOAD on every switch.
# The "natural_log_exp_and_others" set contains BOTH functions; hide Exp/Ln
# from every other set so the chooser must use the combined set, making the
# table resident for the whole kernel.
_orig_get_activation_tables = hw_specs.get_activation_tables


def _patched_get_activation_tables(module_arch):
    tabs = _orig_get_activation_tables(module_arch)
    E = mybir.ActivationFunctionType.Exp
    L = mybir.ActivationFunctionType.Ln
    out = {}
    for name, funcs in tabs.items():
        if name != "natural_log_exp_and_others" and (E in funcs or L in funcs):
            funcs = funcs - {E, L}
        out[name] = funcs
    return out


bacc.get_activation_tables = _patched_get_activation_tables
# ---------------------------------------------------------------------------

BF16 = mybir.dt.bfloat16
FP16 = mybir.dt.float16
FP32 = mybir.dt.float32
FP8 = mybir.dt.float8e4
P = 128  # partitions
W_SCALE = 256.0  # host-side weight_matrix scale before fp8 cast


def _ceil_div(a, b):
    return (a + b - 1) // b


def build_program(n_cores=8, S=2048, D=1024, VSP=6288, KM=2, e_dtype=FP16,
                  use_collectives=True, reps=1, ln_func=None):
    """Build the SPMD Bass program (same program on all cores).

    Inputs (per core):
      hiddenTs [D, S/n] bf16   (this core's token slice)
      w_projT  [D, KM*D] bf16  (same on all cores)
      wgd      [D, 1]   bf16   (w_gate[0] - w_gate[1]; K=2 gate softmax only
                                depends on the logit difference)
      wt       [D, VSP] fp8e4  (core's vocab shard of weight_matrix^T * 256)
      corr     [P, 1]   f32    (number of zero-pad columns in this shard)
    Output (per core):
      out      [S, VSP] fp16
    """
    assert KM == 2, "gate-diff path assumes K=2"
    DC = D // P           # contraction chunks (128 rows each)
    NDP = DC // 2         # DoubleRow pairs (256 rows each)
    ST = S // P           # token tiles
    J = KM * D
    JT = J // P           # projT row tiles
    DR = mybir.MatmulPerfMode.DoubleRow
    # vocab groups, one 4-bank PSUM tile each. Near-even sizes so the short
    # last chunk's matmuls still cover the next DoubleRow LDWEIGHTS (~366ns):
    # a 144-wide-only group would stall PE ~300ns per dpair.
    base = (VSP // 4) // 512 * 512
    groups = []
    v0 = 0
    for g in range(4):
        gw = base if g < 3 else VSP - 3 * base
        groups.append((v0, gw))
        v0 += gw
    assert v0 == VSP and groups[-1][1] <= 2048
    NG = len(groups)
    RG = [list(range(n_cores))]
    if ln_func is None:
        ln_func = mybir.ActivationFunctionType.Ln

    nc = bacc.Bacc(
        "TRN2",
        target_bir_lowering=False,
        debug=False,
        num_devices=n_cores,
    )

    hiddenTs = nc.dram_tensor(
        "hiddenTs", [D, S // n_cores], BF16, kind="ExternalInput"
    ).ap()
    w_projT = nc.dram_tensor("w_projT", [D, J], BF16, kind="ExternalInput").ap()
    wgd = nc.dram_tensor("wgd", [D, 1], BF16, kind="ExternalInput").ap()
    wt = nc.dram_tensor("wt", [D, VSP], FP8, kind="ExternalInput").ap()
    corr = nc.dram_tensor("corr", [P, 1], FP32, kind="ExternalInput").ap()
    out = nc.dram_tensor("out", [S, VSP], FP16, kind="ExternalOutput").ap()

    hts_r = hiddenTs.rearrange("(c p) s -> c p s", p=P)
    wp_r = w_projT.rearrange("(c p) j -> c p j", p=P)
    wgd_r = wgd.rearrange("(c p) one -> c p one", p=P)
    wt_r = wt.rearrange("(c p) v -> c p v", p=P)

    def emit_once(tc):
        with (
            tc.tile_pool(name="singles", bufs=1) as singles,
            tc.tile_pool(name="gates", bufs=2) as gates,
            tc.tile_pool(name="dram", bufs=1, space="DRAM") as dpool,
            tc.tile_pool(name="pj", bufs=3) as pjp,
        ):
            PJ_PRELOAD = 3

            def load_pj(i):
                srow = i * P
                ci = srow // SSH
                soff = srow - ci * SSH
                PJ = pjp.tile([P, JT, P], FP8, tag="PJ", name=f"PJ_{i}")
                nc.sync.dma_start(
                    out=PJ,
                    in_=proj_ag[ci][:, :, soff:soff + P].rearrange(
                        "t p s -> p t s"
                    ),
                )
                return PJ

            # Resident fp8 vocab-shard weights, one tile per 2048-wide group
            # so the first matmuls only wait on their own slice of the load.
            WTs = []
            for gi, (v0, gw) in enumerate(groups):
                wt_tile = singles.tile([P, DC, gw], FP8, tag=f"wt{gi}",
                                       name=f"WT_{gi}")
                for c in range(DC):
                    nc.sync.dma_start(out=wt_tile[:, c, :],
                                      in_=wt_r[c][:, v0:v0 + gw])
                WTs.append(wt_tile)
            corr_sb = singles.tile([P, 1], FP32)
            nc.sync.dma_start(out=corr_sb, in_=corr)

            # Phase 0 is sharded over cores: each core computes projT for
            # S/n_cores tokens, then an AllGather replicates the full projT
            # (in fp8 -- the main loop consumes fp8 anyway).
            SSH = S // n_cores  # tokens per core in phase 0
            assert SSH % P == 0 or n_cores == 1
            proj_in = dpool.tile([JT, P, SSH], FP8, name="proj_in")
            cc_addr = "Shared" if n_cores > 4 else "Local"
            proj_ag = dpool.tile([n_cores, JT, P, SSH], FP8, name="proj_ag",
                                 addr_space=cc_addr)
            gd_in = dpool.tile([1, SSH], FP32, name="gd_in")
            gd_ag = dpool.tile([n_cores, 1, SSH], FP32, name="gd_ag",
                               addr_space=cc_addr)

            # ACT-order chain: order-only edges keep the scalar engine's
            # instruction stream in emission order so Exp/Ln stay batched.
            last_act = [None]

            def act_chain(inst):
                if last_act[0] is not None:
                    tile.add_dep_helper(inst.ins, last_act[0].ins, sync=False,
                                        reason="act table batching")
                last_act[0] = inst
                return inst

            # ---------------- Phase 0: projT = (hidden @ w_proj^T)^T, gate ----
            with (
                tc.tile_pool(name="ph0", bufs=1) as ph0,
                tc.tile_pool(name="ph0ps", bufs=4, space="PSUM") as ps0,
                tc.tile_pool(name="ph0gps", bufs=1, space="PSUM") as gps0,
                tc.tile_pool(name="ph0st", bufs=4) as stg,
            ):
                HTS = ph0.tile([P, DC, SSH], BF16)
                WP = ph0.tile([P, DC, J], BF16)
                WG = ph0.tile([P, DC, 1], BF16)
                for c in range(DC):
                    nc.sync.dma_start(out=HTS[:, c, :], in_=hts_r[c])
                    nc.sync.dma_start(out=WP[:, c, :], in_=wp_r[c])
                    nc.sync.dma_start(out=WG[:, c, :], in_=wgd_r[c])

                # gate-diff gd[s] = sum_d hid[s,d]*(wg0-wg1)[d] for this
                # core's token slice; out lands transposed [1, SSH].
                gp = gps0.tile([1, SSH], FP32)
                for d in range(DC):
                    nc.tensor.matmul(
                        gp,
                        lhsT=WG[:, d, :],
                        rhs=HTS[:, d, :],
                        start=(d == 0),
                        stop=(d == DC - 1),
                    )
                gst = stg.tile([1, SSH], FP32, tag="gst")
                nc.vector.tensor_copy(gst, gp)
                nc.sync.dma_start(out=gd_in, in_=gst)

                # projT[j, s] = sum_d w_projT[d, j] * hiddenT[d, s], for
                # this core's S/n_cores token slice; AllGather replicates.
                pj_tiles = {}
                PSC = min(512, SSH)
                for t in range(JT):
                    for s0 in range(0, SSH, PSC):
                        sw = min(PSC, SSH - s0)
                        psum = ps0.tile([P, PSC], FP32, tag="mm")
                        for d in range(DC):
                            nc.tensor.matmul(
                                psum[:, :sw],
                                lhsT=WP[:, d, t * P:(t + 1) * P],
                                rhs=HTS[:, d, s0:s0 + sw],
                                start=(d == 0),
                                stop=(d == DC - 1),
                            )
                        st = stg.tile([P, PSC], FP8, tag="st")
                        nc.vector.tensor_copy(st[:, :sw], psum[:, :sw])
                        nc.sync.dma_start(out=proj_in[t, :, s0:s0 + sw],
                                          in_=st[:, :sw])
                if use_collectives:
                    nc.gpsimd.collective_compute(
                        "AllGather",
                        mybir.AluOpType.bypass,
                        replica_groups=RG,
                        ins=[gd_in.opt()],
                        outs=[gd_ag.opt()],
                    )
                    nc.gpsimd.collective_compute(
                        "AllGather",
                        mybir.AluOpType.bypass,
                        replica_groups=RG,
                        ins=[proj_in.opt()],
                        outs=[proj_ag.opt()],
                    )
                else:
                    nc.sync.dma_start(out=gd_ag[0], in_=gd_in[:])
                    nc.sync.dma_start(out=proj_ag[0], in_=proj_in[:])
                # Prefetch the first main-loop lhsT slices now so their
                # DMAs aren't queued behind the rest of phase 0.
                for i in range(min(PJ_PRELOAD, ST)):
                    pj_tiles[i] = load_pj(i)

                # x[p, i] = exp(-(g0-g1)) for token i*128+p, all tiles at once
                gdT = gates.tile([P, ST], FP32)
                nc.sync.dma_start(
                    out=gdT,
                    in_=gd_ag.rearrange("c one (i p) -> p (one c i)", p=P),
                )
                xall = gates.tile([P, ST], FP32)
                act_chain(nc.scalar.activation(
                    out=xall, in_=gdT,
                    func=mybir.ActivationFunctionType.Exp,
                    scale=-1.0,
                ))

            # ---------------- Main loop over token tiles ----------------------
            with (
                tc.tile_pool(name="ebuf", bufs=3) as ep,
                tc.tile_pool(name="zp", bufs=3) as zpp,
                tc.tile_pool(name="mmps", bufs=2, space="PSUM") as psm,
                tc.tile_pool(name="ocp", bufs=2) as ocp,
                tc.tile_pool(name="ttp", bufs=2) as ttp,
                tc.tile_pool(name="s2", bufs=3) as s2p,
                tc.tile_pool(name="cc", bufs=2 * ST, space="DRAM") as ccp,
            ):
                def emit_exps(i, k, E, zpart, PJ):
                    for g, (v0, gw) in enumerate(groups):
                        ps = psm.tile([P, 2048], FP32, tag="mm")
                        nchunks = _ceil_div(gw, 512)
                        for j in range(NDP):
                            lhsT = PJ[:, k * DC + 2 * j:k * DC + 2 * j + 2, :]
                            for c in range(nchunks):
                                cw = min(512, gw - c * 512)
                                nc.tensor.matmul(
                                    ps[:, c * 512:c * 512 + cw],
                                    lhsT=lhsT,
                                    rhs=WTs[g][:, 2 * j:2 * j + 2,
                                               c * 512:c * 512 + cw],
                                    start=(j == 0),
                                    stop=(j == NDP - 1),
                                    perf_mode=DR,
                                )
                        act_chain(nc.scalar.activation(
                            out=E[:, k, v0:v0 + gw],
                            in_=ps[:, :gw],
                            func=mybir.ActivationFunctionType.Exp,
                            scale=1.0 / W_SCALE,
                            accum_out=zpart[:, k, g:g + 1],
                        ))

                def emit_stage2(i, E, Zg):
                    srow = i * P
                    # x = e^{-(g0-g1)}: pi0 = 1/(1+x), pi1 = x/(1+x)
                    # w_k = pi_k/Z_k; r01 = w0/w1 = Z1/(x*Z0);
                    # w1 = x/((1+x)*Z1)
                    x = xall[:, i:i + 1]
                    xp1 = s2p.tile([P, 1], FP32, tag="xp1")
                    nc.vector.tensor_scalar_add(xp1, x, 1.0)
                    m = s2p.tile([P, 1], FP32, tag="m")
                    nc.vector.tensor_mul(m, x, Zg[:, 0:1])
                    rm = s2p.tile([P, 1], FP32, tag="rm")
                    nc.vector.reciprocal(rm, m)
                    r01 = s2p.tile([P, 1], FP32, tag="r01")
                    nc.vector.tensor_mul(r01, rm, Zg[:, 1:2])
                    n = s2p.tile([P, 1], FP32, tag="n")
                    nc.vector.tensor_mul(n, xp1, Zg[:, 1:2])
                    rn = s2p.tile([P, 1], FP32, tag="rn")
                    nc.vector.reciprocal(rn, n)
                    w1 = s2p.tile([P, 1], FP32, tag="w1")
                    nc.vector.tensor_mul(w1, x, rn)
                    # t = E0 * (w0/w1) + E1, one fused DVE pass in fp16
                    t = ttp.tile([P, VSP], FP16, tag="t")
                    nc.vector.scalar_tensor_tensor(
                        out=t,
                        in0=E[:, 0, :],
                        scalar=r01,
                        in1=E[:, 1, :],
                        op0=mybir.AluOpType.mult,
                        op1=mybir.AluOpType.add,
                    )
                    oc = ocp.tile([P, VSP], FP16, tag="oc")
                    act_chain(nc.scalar.activation(
                        out=oc,
                        in_=t,
                        func=ln_func,
                        scale=w1,
                    ))
                    nc.sync.dma_start(out=out[srow:srow + P, :], in_=oc)

                pending = []  # [(i, E, Zg)] awaiting stage 2 (depth 2)
                for i in range(ST):
                    if i not in pj_tiles:
                        pj_tiles[i] = load_pj(i)
                    nxt = i + PJ_PRELOAD
                    if nxt < ST and nxt not in pj_tiles:
                        pj_tiles[nxt] = load_pj(nxt)
                    PJ = pj_tiles.pop(i)
                    E = ep.tile([P, KM, VSP], e_dtype)
                    zpart = zpp.tile([P, KM, NG], FP32)
                    emit_exps(i, 0, E, zpart, PJ)
                    if len(pending) >= 2:
                        emit_stage2(*pending.pop(0))
                    for k in range(1, KM):
                        emit_exps(i, k, E, zpart, PJ)
                    zloc = s2p.tile([P, KM], FP32, tag="zloc")
                    for k in range(KM):
                        nc.vector.reduce_sum(
                            out=zloc[:, k:k + 1],
                            in_=zpart[:, k, :],
                            axis=mybir.AxisListType.X,
                        )
                    # remove pad-column contribution (exp(0)=1 per pad col)
                    nc.vector.tensor_scalar_sub(zloc, zloc, corr_sb)
                    if i == ST - 1 and pending:
                        # shrink the drain tail: stage2(ST-2) can run while
                        # tile ST-1's AllReduce is in flight
                        emit_stage2(*pending.pop(0))

                    cin = ccp.tile([P, KM], FP32, tag="cin")
                    cout = ccp.tile([P, KM], FP32, tag="cout",
                                    addr_space=cc_addr)
                    nc.sync.dma_start(out=cin, in_=zloc)
                    if use_collectives:
                        nc.gpsimd.collective_compute(
                            "AllReduce",
                            mybir.AluOpType.add,
                            replica_groups=RG,
                            ins=[cin.opt()],
                            outs=[cout.opt()],
                        )
                    else:
                        nc.sync.dma_start(out=cout, in_=cin)
                    Zg = s2p.tile([P, KM], FP32, tag="zg")
                    nc.sync.dma_start(out=Zg, in_=cout)
                    pending.append((i, E, Zg))
                while pending:
                    emit_stage2(*pending.pop(0))

    with tile.TileContext(nc) as tc:
        for _ in range(reps):
            emit_once(tc)

    nc.compile()
    return nc


def prep_inputs(hidden, weight_matrix, w_proj, w_gate, n_cores=8):
    """Host-side shard/transpose/cast. Returns (in_maps, VS, VSP)."""
    bf16 = ml_dtypes.bfloat16
    fp8 = ml_dtypes.float8_e4m3
    B, S, D = hidden.shape
    V = weight_matrix.shape[0]
    VS = _ceil_div(V, n_cores)       # logical shard width (6283)
    VSP = _ceil_div(VS, 16) * 16     # on-chip width, multiple of 16 (6288)

    hiddenT = np.ascontiguousarray(
        np.asarray(hidden, dtype=np.float32).reshape(S, D).T
    ).astype(bf16)
    w_projT = np.ascontiguousarray(
        np.asarray(w_proj, dtype=np.float32).T
    ).astype(bf16)
    wg = np.asarray(w_gate, dtype=np.float32)
    wgd = np.ascontiguousarray((wg[0] - wg[1]).reshape(D, 1)).astype(bf16)

    wmat = np.asarray(weight_matrix, dtype=np.float32)
    SSH = S // n_cores
    in_maps = []
    for c in range(n_cores):
        lo = c * VS
        hi = min(lo + VS, V)
        shard = np.zeros((VSP, D), dtype=np.float32)
        shard[: hi - lo] = wmat[lo:hi]
        wt_c = np.clip(
            np.ascontiguousarray(shard.T) * W_SCALE, -240.0, 240.0
        ).astype(fp8)
        npad = VSP - (hi - lo)
        corr_c = np.full((P, 1), float(npad), dtype=np.float32)
        in_maps.append(
            {
                "hiddenTs": np.ascontiguousarray(
                    hiddenT[:, c * SSH:(c + 1) * SSH]
                ),
                "w_projT": w_projT,
                "wgd": wgd,
                "wt": wt_c,
                "corr": corr_c,
            }
        )
    return in_maps, VS, VSP


_PROGRAM_CACHE = {}


def kernel(hidden, weight_matrix, w_proj, w_gate):
    import time

    n_cores = 8
    B, S, D = hidden.shape
    V = weight_matrix.shape[0]
    KM = w_gate.shape[0]
    in_maps, VS, VSP = prep_inputs(hidden, weight_matrix, w_proj, w_gate,
                                   n_cores)

    key = (n_cores, S, D, VSP, KM)
    if key not in _PROGRAM_CACHE:
        _PROGRAM_CACHE[key] = build_program(n_cores, S, D, VSP, KM)
    nc = _PROGRAM_CACHE[key]

    # The axon terminal occasionally reports a transient
    # NRT_EXEC_UNIT_UNRECOVERABLE right after another process released the
    # devices; one retry after a pause usually succeeds.
    last_err = None
    for attempt in range(2):
        try:
            res = run_bass_kernel_spmd(nc, in_maps, core_ids=list(range(n_cores)))
            break
        except Exception as e:  # noqa: BLE001
            last_err = e
            time.sleep(15)
    else:
        raise last_err

    full = np.empty((S, VS * n_cores), dtype=np.float32)
    for c in range(n_cores):
        full[:, c * VS:(c + 1) * VS] = res.results[c]["out"][:, :VS]
    return full[:, :V].reshape(B, S, V)
